# revision 9
# baseline (speedup 1.0000x reference)
"""Trainium2 Bass kernel for CropSplit (SipMask-style crop + quadrant split).

Reference computation, per output pixel (y, x, n):
    inside = point (x, y) lies in box rois[n] = (x1, y1, x2, y2)
    cell   = which of the 2x2 ROI sub-cells the pixel falls in
    out[y, x, n] = inside ? data[cell, y, x, n] : 0

Strategy (v2 — DMA-engine balanced):
  - Shard along W across the 8 cores (25 columns each); tile layout
    [h -> partitions, (cc, w, n) -> free] so every DMA row is a large
    contiguous DRAM block.
  - Each of the 16 SDMA engines serves a fixed group of 8 SBUF
    partitions, so per-engine bytes are set by how rows map to
    partitions.  H=200 rows are mapped:
      rows   0..127 -> partitions p = h          (stride 1: 8 rows/group)
      rows 128..191 -> partitions p = 2(h-128)   (stride 2: 4 rows/group)
      rows 192..199 -> partitions p = s + 16(h-192), s alternating 1/9
                       per w-block (1 row on even/odd groups alternately)
    => every engine group moves ~12.5 rows instead of the 8/16 split a
    naive 128+72 chunking gives (that imbalance capped the old kernel at
    ~75% DMA utilization in its second half).
  - Masks are computed on host in float32 with bit-identical arithmetic
    to the reference and shipped as uint8 packed in the same row layout:
        nin[p, hpart, w, n] = NOT inside   (zeroing mask)
        my[p, hpart, n]     = (cy == 1)    (quadrant row select)
    The x-mask hx[w, n] is identical on every partition, so only one
    partition's copy is shipped (5 KB) and gpsimd.partition_broadcast
    fans it out across all 128 partitions on-device (saves 640 KB of
    HBM traffic vs shipping it pre-broadcast).
  - Per tile, the 4-way select + mask is 3 predicated DVE ops:
        cp(dall[0::2], hx, dall[1::2])  (d0<-d1, d2<-d3 where cx==1)
        cp(dall[0], my, dall[2])        (y-blend -> 4-way select)
        cp(dall[0], nin, 0)             (zero outside the box)
    w-blocks are [7,7,6,4,1] (few big ops to amortize DVE op overhead,
    tiny last block so the drain tail is short).
  - Op order is y-blend -> x-blend -> zero: the y/zero masks arrive in
    the first (tiny) DMAs, so the DVE can start y-blending tiles while
    the gpsimd partition_broadcast of the x-mask (~8us) is still in
    flight; with several tiles buffered the broadcast latency hides
    completely.
  - All 4 cc planes of a tile arrive in one DMA; DMA issue is split
    across both HWDGE sequencers (Sync for data loads, Scalar for
    masks/stores).
"""

import numpy as np

C = 2
CC = C * C
H = W = N = 200
NCORES = 8
WS = W // NCORES  # 25 columns per core

# w-blocks: (w0, wb). Small first (fast pipeline ramp: the DVE can
# start on a small tile ~3us after DMA start instead of ~9), big middle
# (amortize DVE op overhead), small last (short drain tail).
W_BLOCKS = [(0, 2), (2, 5), (7, 7), (14, 6), (20, 4), (24, 1)]
# per-block sigma for the 8-row tail (rows 192..199): partitions
# sigma+16j. Alternating 1/9 spreads the tail rows' bytes over even and
# odd engine groups so per-engine totals stay balanced (even-parity
# blocks carry 13 columns, odd-parity 12).
SIGMA = [1, 9, 1, 9, 1, 9]
DATA_BUFS = 6

_cache: dict = {}


def _row_of_partition():
    """row_map[block_parity][hpart][p] = source row h for partition p (or -1)."""
    maps = []
    for sigma in (1, 9):
        m = np.full((2, 128), -1, dtype=np.int64)
        m[0, :] = np.arange(128)  # hpart 0: rows 0..127
        m[1, 0:128:2] = 128 + np.arange(64)  # hpart 1: rows 128..191
        m[1, sigma:sigma + 16 * 8:16] = 192 + np.arange(8)  # rows 192..199
        maps.append(m)
    return maps


_ROWMAPS = _row_of_partition()


def _build_module():
    import concourse.bacc as bacc
    import concourse.mybir as mybir
    from concourse.tile import TileContext

    f32 = mybir.dt.float32
    u8 = mybir.dt.uint8

    nc = bacc.Bacc(trn_type="TRN2", debug=False, num_devices=NCORES)
    data = nc.dram_tensor("data", [CC, H, WS, N], f32, kind="ExternalInput")
    # hx (quadrant column select) for this core's 25 columns: one copy.
    mx1 = nc.dram_tensor("mx1", [1, WS, N], u8, kind="ExternalInput")
    # not-inside mask packed per (sigma-parity, hpart): [p, par, hpart, w, n]
    ninb = nc.dram_tensor("ninb", [128, 2, 2, WS, N], u8, kind="ExternalInput")
    # y-select mask packed the same way: [p, par, hpart, n]
    myb = nc.dram_tensor("myb", [128, 2, 2, N], u8, kind="ExternalInput")
    out = nc.dram_tensor("out", [H, WS, N], f32, kind="ExternalOutput")

    with TileContext(nc) as tc:
        with (
            tc.tile_pool(name="masks", bufs=1) as mpool,
            tc.tile_pool(name="dpool", bufs=DATA_BUFS) as dpool,
        ):
            zeros = mpool.tile([128, 1], f32)
            nc.vector.memset(zeros[:], 0.0)

            # x-mask: first load issued (sync queue, 5 KB) so the slow
            # gpsimd broadcast starts as early as possible.
            mx_row = mpool.tile([1, WS, N], u8, tag="mxrow")
            nc.sync.dma_start(mx_row[:], mx1[:])
            mxt = mpool.tile([128, WS, N], u8, tag="mxb")
            nc.gpsimd.partition_broadcast(mxt[:], mx_row[:])

            # y-masks in one small load: [128, 2, 2, N]
            ymask = mpool.tile([128, 2, 2, N], u8)
            nc.scalar.dma_start(ymask[:], myb[:])

            for bi, (w0, wb) in enumerate(W_BLOCKS):
                par = 0 if SIGMA[bi] == 1 else 1
                for hp in range(2):
                    # all 4 cell planes in one tile; loaded by one DMA
                    # (hp 0) or three DMAs into disjoint partition sets
                    # (hp 1) so every engine group stays fed.
                    dall = dpool.tile([128, CC, wb, N], f32, tag="dall")
                    src = data[:, :, w0 : w0 + wb, :]
                    if hp == 0:
                        nc.sync.dma_start(
                            dall[:],
                            src[:, 0:128].transpose([1, 0, 2, 3]),
                        )
                    else:
                        nc.sync.dma_start(
                            dall[0:128:2],
                            src[:, 128:192].transpose([1, 0, 2, 3]),
                        )
                        s = SIGMA[bi]
                        nc.sync.dma_start(
                            dall[s : s + 113 : 16],
                            src[:, 192:200].transpose([1, 0, 2, 3]),
                        )
                    t_nin = dpool.tile([128, wb, N], u8, tag="nin")
                    nc.scalar.dma_start(
                        t_nin[:], ninb[:, par, hp, w0 : w0 + wb, :]
                    )
                    hyv2 = ymask[:, par, hp, None, None, :].broadcast_to(
                        (128, 2, wb, N)
                    )
                    hxv = mxt[:, w0 : w0 + wb, :]
                    zv = zeros[:, :, None].broadcast_to((128, wb, N))
                    # y-blend both cell columns in one op (planes 0,1 <-
                    # 2,3 where cy==1), then x-blend, then zero.
                    nc.vector.copy_predicated(
                        dall[:, 0:2], hyv2, dall[:, 2:4]
                    )
                    nc.vector.copy_predicated(dall[:, 0], hxv, dall[:, 1])
                    nc.vector.copy_predicated(dall[:, 0], t_nin[:], zv)
                    if hp == 0:
                        nc.scalar.dma_start(
                            out[0:128, w0 : w0 + wb, :], dall[0:128, 0]
                        )
                    else:
                        nc.scalar.dma_start(
                            out[128:192, w0 : w0 + wb, :], dall[0:128:2, 0]
                        )
                        s = SIGMA[bi]
                        nc.scalar.dma_start(
                            out[192:200, w0 : w0 + wb, :],
                            dall[s : s + 113 : 16, 0],
                        )
    nc.finalize()
    return nc


def _get_module():
    if "nc" not in _cache:
        _cache["nc"] = _build_module()
    return _cache["nc"]


def _host_masks(rois):
    """Masks in f32 arithmetic bit-identical to the reference, as uint8."""
    r = np.asarray(rois, dtype=np.float32)
    x1, y1, x2, y2 = r[:, 0], r[:, 1], r[:, 2], r[:, 3]
    two = np.float32(2.0)
    one = np.float32(1.0)

    xs = np.arange(W, dtype=np.float32)[:, None]  # (W, 1)
    cw = np.maximum(x2 - x1, one)[None, :]  # (1, N)
    fx = np.floor(two * (xs - x1[None, :]) / cw)
    hx = (fx >= 1.0).astype(np.uint8)  # clip(floor, 0, 1) == 1
    nix = (~((xs >= x1[None, :]) & (xs <= x2[None, :]))).astype(np.uint8)

    ys = np.arange(H, dtype=np.float32)[:, None]  # (H, 1)
    ch = np.maximum(y2 - y1, one)[None, :]
    fy = np.floor(two * (ys - y1[None, :]) / ch)
    hy = (fy >= 1.0).astype(np.uint8)
    niy = (~((ys >= y1[None, :]) & (ys <= y2[None, :]))).astype(np.uint8)

    return hx, nix, hy, niy


def _pack_rows(arr_by_row, fill):
    """arr_by_row: (H, ...) -> packed (128, 2, 2, ...) per (parity, hpart)."""
    shp = (128, 2, 2) + arr_by_row.shape[1:]
    outp = np.full(shp, fill, dtype=arr_by_row.dtype)
    for par in range(2):
        rm = _ROWMAPS[par]
        for hp in range(2):
            valid = rm[hp] >= 0
            outp[valid, par, hp] = arr_by_row[rm[hp][valid]]
    return outp


def _run(data, rois, trace=False):
    from concourse.bass_utils import run_bass_kernel_spmd

    data = np.ascontiguousarray(np.asarray(data, dtype=np.float32))
    hx, nix, hy, niy = _host_masks(rois)

    # y masks packed per (parity, hpart): [128, 2, 2, N]
    myb = _pack_rows(hy, 0)

    in_maps = []
    for i in range(NCORES):
        sl = slice(i * WS, (i + 1) * WS)
        # not-inside per pixel: nix(w,n) OR niy(h,n)  -> (H, WS, N)
        nin = np.maximum(nix[sl, :][None, :, :], niy[:, None, :])
        ninb = _pack_rows(nin, 1)
        in_maps.append(
            {
                "data": np.ascontiguousarray(data[:, :, sl, :]),
                "mx1": np.ascontiguousarray(hx[sl, :][None]),
                "ninb": np.ascontiguousarray(ninb),
                "myb": np.ascontiguousarray(myb),
            }
        )

    nc = _get_module()
    last_err = None
    for _attempt in range(2):
        try:
            res = run_bass_kernel_spmd(
                nc, in_maps, core_ids=list(range(NCORES)), trace=trace
            )
            break
        except Exception as e:  # transient NRT device errors: retry once
            last_err = e
    else:
        raise last_err
    full = np.concatenate([r["out"] for r in res.results], axis=1)
    return np.asarray(full, dtype=np.float32), res


def kernel(data, rois):
    out, _ = _run(data, rois, trace=False)
    return out


# revision 10
# speedup vs baseline: 1.0173x; 1.0173x over previous
"""Trainium2 Bass kernel for CropSplit (SipMask-style crop + quadrant split).

Reference computation, per output pixel (y, x, n):
    inside = point (x, y) lies in box rois[n] = (x1, y1, x2, y2)
    cell   = which of the 2x2 ROI sub-cells the pixel falls in
    out[y, x, n] = inside ? data[cell, y, x, n] : 0

Strategy (v2 — DMA-engine balanced):
  - Shard along W across the 8 cores (25 columns each); tile layout
    [h -> partitions, (cc, w, n) -> free] so every DMA row is a large
    contiguous DRAM block.
  - Each of the 16 SDMA engines serves a fixed group of 8 SBUF
    partitions, so per-engine bytes are set by how rows map to
    partitions.  H=200 rows are mapped:
      rows   0..127 -> partitions p = h          (stride 1: 8 rows/group)
      rows 128..191 -> partitions p = 2(h-128)   (stride 2: 4 rows/group)
      rows 192..199 -> partitions p = s + 16(h-192), s alternating 1/9
                       per w-block (1 row on even/odd groups alternately)
    => every engine group moves ~12.5 rows instead of the 8/16 split a
    naive 128+72 chunking gives (that imbalance capped the old kernel at
    ~75% DMA utilization in its second half).
  - Masks are computed on host in float32 with bit-identical arithmetic
    to the reference and shipped as uint8 packed in the same row layout:
        nin[p, hpart, w, n] = NOT inside   (zeroing mask)
        my[p, hpart, n]     = (cy == 1)    (quadrant row select)
    The x-mask hx[w, n] is identical on every partition, so only one
    partition's copy is shipped (5 KB) and gpsimd.partition_broadcast
    fans it out across all 128 partitions on-device (saves 640 KB of
    HBM traffic vs shipping it pre-broadcast).
  - Per tile, the 4-way select + mask is 3 predicated DVE ops:
        cp(dall[0::2], hx, dall[1::2])  (d0<-d1, d2<-d3 where cx==1)
        cp(dall[0], my, dall[2])        (y-blend -> 4-way select)
        cp(dall[0], nin, 0)             (zero outside the box)
    w-blocks are [7,7,6,4,1] (few big ops to amortize DVE op overhead,
    tiny last block so the drain tail is short).
  - Op order is y-blend -> x-blend -> zero: the y/zero masks arrive in
    the first (tiny) DMAs, so the DVE can start y-blending tiles while
    the gpsimd partition_broadcast of the x-mask (~8us) is still in
    flight; with several tiles buffered the broadcast latency hides
    completely.
  - All 4 cc planes of a tile arrive in one DMA; DMA issue is split
    across both HWDGE sequencers (Sync for data loads, Scalar for
    masks/stores).
"""

import numpy as np

C = 2
CC = C * C
H = W = N = 200
NCORES = 8
WS = W // NCORES  # 25 columns per core

# w-blocks: (w0, wb). Small first (fast pipeline ramp: the DVE can
# start on a small tile ~3us after DMA start instead of ~9), big middle
# (amortize DVE op overhead), small last (short drain tail).
W_BLOCKS = [(0, 2), (2, 5), (7, 7), (14, 6), (20, 4), (24, 1)]
# per-block sigma for the 8-row tail (rows 192..199): partitions
# sigma+16j. Alternating 1/9 spreads the tail rows' bytes over even and
# odd engine groups so per-engine totals stay balanced (even-parity
# blocks carry 13 columns, odd-parity 12).
SIGMA = [1, 9, 1, 9, 1, 9]
DATA_BUFS = 6

_cache: dict = {}


def _row_of_partition():
    """row_map[block_parity][hpart][p] = source row h for partition p (or -1)."""
    maps = []
    for sigma in (1, 9):
        m = np.full((2, 128), -1, dtype=np.int64)
        m[0, :] = np.arange(128)  # hpart 0: rows 0..127
        m[1, 0:128:2] = 128 + np.arange(64)  # hpart 1: rows 128..191
        m[1, sigma:sigma + 16 * 8:16] = 192 + np.arange(8)  # rows 192..199
        maps.append(m)
    return maps


_ROWMAPS = _row_of_partition()


def _build_module():
    import concourse.bacc as bacc
    import concourse.mybir as mybir
    from concourse.tile import TileContext

    f32 = mybir.dt.float32
    u8 = mybir.dt.uint8

    nc = bacc.Bacc(trn_type="TRN2", debug=False, num_devices=NCORES)
    data = nc.dram_tensor("data", [CC, H, WS, N], f32, kind="ExternalInput")
    # hx (quadrant column select) for this core's 25 columns: one copy.
    mx1 = nc.dram_tensor("mx1", [1, WS, N], u8, kind="ExternalInput")
    # not-inside mask packed per (sigma-parity, hpart): [p, par, hpart, w, n]
    ninb = nc.dram_tensor("ninb", [128, 2, 2, WS, N], u8, kind="ExternalInput")
    # y-select mask packed the same way: [p, par, hpart, n]
    myb = nc.dram_tensor("myb", [128, 2, 2, N], u8, kind="ExternalInput")
    out = nc.dram_tensor("out", [H, WS, N], f32, kind="ExternalOutput")

    with TileContext(nc) as tc:
        with (
            tc.tile_pool(name="masks", bufs=1) as mpool,
            tc.tile_pool(name="dpool", bufs=DATA_BUFS) as dpool,
        ):
            zeros = mpool.tile([128, 1], f32)
            nc.vector.memset(zeros[:], 0.0)

            # x-mask: first load issued (sync queue, 5 KB), then fanned
            # out across partitions on gpsimd.  One broadcast per w-block
            # (not one big one): the DVE is in-order, so block0's x-blend
            # must not wait on the full 7.5us broadcast — the per-block
            # pieces complete just ahead of each block's x-blend.
            mx_row = mpool.tile([1, WS, N], u8, tag="mxrow")
            nc.sync.dma_start(mx_row[:], mx1[:])
            mxt = mpool.tile([128, WS, N], u8, tag="mxb")
            for w0, wb in W_BLOCKS:
                nc.gpsimd.partition_broadcast(
                    mxt[:, w0 : w0 + wb, :], mx_row[:, w0 : w0 + wb, :]
                )

            # y-masks in one small load: [128, 2, 2, N]
            ymask = mpool.tile([128, 2, 2, N], u8)
            nc.scalar.dma_start(ymask[:], myb[:])

            for bi, (w0, wb) in enumerate(W_BLOCKS):
                par = 0 if SIGMA[bi] == 1 else 1
                for hp in range(2):
                    # all 4 cell planes in one tile; loaded by one DMA
                    # (hp 0) or three DMAs into disjoint partition sets
                    # (hp 1) so every engine group stays fed.
                    dall = dpool.tile([128, CC, wb, N], f32, tag="dall")
                    src = data[:, :, w0 : w0 + wb, :]
                    if hp == 0:
                        nc.sync.dma_start(
                            dall[:],
                            src[:, 0:128].transpose([1, 0, 2, 3]),
                        )
                    else:
                        nc.sync.dma_start(
                            dall[0:128:2],
                            src[:, 128:192].transpose([1, 0, 2, 3]),
                        )
                        s = SIGMA[bi]
                        nc.sync.dma_start(
                            dall[s : s + 113 : 16],
                            src[:, 192:200].transpose([1, 0, 2, 3]),
                        )
                    t_nin = dpool.tile([128, wb, N], u8, tag="nin")
                    nc.scalar.dma_start(
                        t_nin[:], ninb[:, par, hp, w0 : w0 + wb, :]
                    )
                    hyv2 = ymask[:, par, hp, None, None, :].broadcast_to(
                        (128, 2, wb, N)
                    )
                    hxv = mxt[:, w0 : w0 + wb, :]
                    zv = zeros[:, :, None].broadcast_to((128, wb, N))
                    # y-blend both cell columns in one op (planes 0,1 <-
                    # 2,3 where cy==1), then x-blend, then zero.
                    nc.vector.copy_predicated(
                        dall[:, 0:2], hyv2, dall[:, 2:4]
                    )
                    nc.vector.copy_predicated(dall[:, 0], hxv, dall[:, 1])
                    nc.vector.copy_predicated(dall[:, 0], t_nin[:], zv)
                    if hp == 0:
                        nc.scalar.dma_start(
                            out[0:128, w0 : w0 + wb, :], dall[0:128, 0]
                        )
                    else:
                        nc.scalar.dma_start(
                            out[128:192, w0 : w0 + wb, :], dall[0:128:2, 0]
                        )
                        s = SIGMA[bi]
                        nc.scalar.dma_start(
                            out[192:200, w0 : w0 + wb, :],
                            dall[s : s + 113 : 16, 0],
                        )
    nc.finalize()
    return nc


def _get_module():
    if "nc" not in _cache:
        _cache["nc"] = _build_module()
    return _cache["nc"]


def _host_masks(rois):
    """Masks in f32 arithmetic bit-identical to the reference, as uint8."""
    r = np.asarray(rois, dtype=np.float32)
    x1, y1, x2, y2 = r[:, 0], r[:, 1], r[:, 2], r[:, 3]
    two = np.float32(2.0)
    one = np.float32(1.0)

    xs = np.arange(W, dtype=np.float32)[:, None]  # (W, 1)
    cw = np.maximum(x2 - x1, one)[None, :]  # (1, N)
    fx = np.floor(two * (xs - x1[None, :]) / cw)
    hx = (fx >= 1.0).astype(np.uint8)  # clip(floor, 0, 1) == 1
    nix = (~((xs >= x1[None, :]) & (xs <= x2[None, :]))).astype(np.uint8)

    ys = np.arange(H, dtype=np.float32)[:, None]  # (H, 1)
    ch = np.maximum(y2 - y1, one)[None, :]
    fy = np.floor(two * (ys - y1[None, :]) / ch)
    hy = (fy >= 1.0).astype(np.uint8)
    niy = (~((ys >= y1[None, :]) & (ys <= y2[None, :]))).astype(np.uint8)

    return hx, nix, hy, niy


def _pack_rows(arr_by_row, fill):
    """arr_by_row: (H, ...) -> packed (128, 2, 2, ...) per (parity, hpart)."""
    shp = (128, 2, 2) + arr_by_row.shape[1:]
    outp = np.full(shp, fill, dtype=arr_by_row.dtype)
    for par in range(2):
        rm = _ROWMAPS[par]
        for hp in range(2):
            valid = rm[hp] >= 0
            outp[valid, par, hp] = arr_by_row[rm[hp][valid]]
    return outp


def _run(data, rois, trace=False):
    from concourse.bass_utils import run_bass_kernel_spmd

    data = np.ascontiguousarray(np.asarray(data, dtype=np.float32))
    hx, nix, hy, niy = _host_masks(rois)

    # y masks packed per (parity, hpart): [128, 2, 2, N]
    myb = _pack_rows(hy, 0)

    in_maps = []
    for i in range(NCORES):
        sl = slice(i * WS, (i + 1) * WS)
        # not-inside per pixel: nix(w,n) OR niy(h,n)  -> (H, WS, N)
        nin = np.maximum(nix[sl, :][None, :, :], niy[:, None, :])
        ninb = _pack_rows(nin, 1)
        in_maps.append(
            {
                "data": np.ascontiguousarray(data[:, :, sl, :]),
                "mx1": np.ascontiguousarray(hx[sl, :][None]),
                "ninb": np.ascontiguousarray(ninb),
                "myb": np.ascontiguousarray(myb),
            }
        )

    nc = _get_module()
    last_err = None
    for _attempt in range(2):
        try:
            res = run_bass_kernel_spmd(
                nc, in_maps, core_ids=list(range(NCORES)), trace=trace
            )
            break
        except Exception as e:  # transient NRT device errors: retry once
            last_err = e
    else:
        raise last_err
    full = np.concatenate([r["out"] for r in res.results], axis=1)
    return np.asarray(full, dtype=np.float32), res


def kernel(data, rois):
    out, _ = _run(data, rois, trace=False)
    return out
